# revision 1
# baseline (speedup 1.0000x reference)
"""PIoU (pixel-wise IoU) pairwise matrix kernel for Trainium2, 8 NeuronCores.

Math: for each pair (predicted box n, target box m) the reference samples a
16x16 grid of the joint AABB and evaluates a soft membership
F = sigmoid(k(w/2-|A|)) * sigmoid(k(h/2-|B|)) per box, where (A, B) are the
pixel offsets rotated into the box frame.  Both A and B are *affine* in the
grid coordinates (ug, uh), so the sigmoid arguments k(s/2 -+ A) for all
256 pixels x 4 fields x {P,Q} are produced by ONE K=24 matmul per 128 pairs
against a constant basis.  sigmoid(min(P,Q)) == min(sigmoid(P), sigmoid(Q))
lets ACT read the matmul PSUM directly with no bias work.

Sharding: N (predicted) axis split 8 ways; each core computes a [512m, 64n]
slab (output transposed on host).
"""

import numpy as np

N = 512
M = 512
G = 16
NPIX = G * G
K_SLOPE = np.float32(10.0)
EPS = np.float32(1e-6)
NC = 8
NLOC = N // NC  # 64 predicted boxes per core
NCHUNK = 4  # m-chunks of 128

_cache = {}


def _derived(b):
    # b: [K,5] float32 -> per-box derived quantities (all float32)
    cx, cy, w, h, t = (b[:, i].astype(np.float32) for i in range(5))
    c, s = np.cos(t).astype(np.float32), np.sin(t).astype(np.float32)
    hw = np.float32(0.5) * (w * np.abs(c) + h * np.abs(s))
    hh = np.float32(0.5) * (w * np.abs(s) + h * np.abs(c))
    return dict(
        cx=cx, cy=cy, ct=c, st=s,
        khw=(K_SLOPE * np.float32(0.5)) * w, khh=(K_SLOPE * np.float32(0.5)) * h,
        x0=cx - hw, x1=cx + hw, y0=cy - hh, y1=cy + hh,
    )


def _host_constants(loc_p, loc_t):
    """Build per-core input arrays (all O(N+M) host work)."""
    u = ((np.arange(G, dtype=np.float32) + np.float32(0.5)) / np.float32(G))
    Ug = np.tile(u, G)      # pixel p = h*G+g -> u[g]
    Uh = np.repeat(u, G)    # -> u[h]

    # basis [24, 2*NPIX*4]: P-block cols 0..1023 (fields A1,B1,A2,B2 x 256),
    # Q-block cols 1024..2047.  Field f uses rows 3f..3f+2 (P) / 12+3f.. (Q).
    basis = np.zeros((24, 8 * NPIX), dtype=np.float32)
    for f in range(4):
        for blk, r0 in ((0, 0), (1, 12)):
            c0 = blk * 4 * NPIX + f * NPIX
            basis[r0 + 3 * f + 0, c0:c0 + NPIX] = 1.0
            basis[r0 + 3 * f + 1, c0:c0 + NPIX] = Ug
            basis[r0 + 3 * f + 2, c0:c0 + NPIX] = Uh

    T = _derived(loc_t)
    # TQ [128, 4 chunks, 10]: per-target quantities, m = j*128 + partition
    tq_order = ("x0", "x1", "y0", "y1", "cx", "cy", "ct", "st", "khw", "khh")
    TQ = np.empty((128, NCHUNK, len(tq_order)), dtype=np.float32)
    for qi, q in enumerate(tq_order):
        TQ[:, :, qi] = T[q].reshape(NCHUNK, 128).T

    P = _derived(loc_p)
    pb_order = ("x0", "x1", "y0", "y1", "cx", "cy", "ct", "st", "khw", "khh")
    PBs = []
    for c in range(NC):
        sl = slice(c * NLOC, (c + 1) * NLOC)
        pb = np.stack([P[q][sl] for q in pb_order], axis=0)  # [10, 64]
        PBs.append(np.broadcast_to(pb.reshape(1, 10 * NLOC), (128, 10 * NLOC)).copy())
    return basis, TQ.reshape(128, NCHUNK * len(tq_order)), PBs


def _build_nc():
    from contextlib import ExitStack

    import concourse.bacc as bacc
    import concourse.tile as tile
    from concourse import mybir
    from concourse.masks import make_identity

    dt = mybir.dt
    op = mybir.AluOpType
    AF = mybir.ActivationFunctionType
    K = float(K_SLOPE)

    # Bacc (not raw Bass): its finalize() runs generate_event_semaphores,
    # which legalizes Tile's multi-wait sync_info down to <=1 wait per
    # hardware instruction.
    nc = bacc.Bacc(None, target_bir_lowering=False)
    PB_d = nc.declare_dram_parameter("PB", [128, 10 * NLOC], dt.float32, isOutput=False)
    TQ_d = nc.declare_dram_parameter("TQ", [128, NCHUNK * 10], dt.float32, isOutput=False)
    BAS_d = nc.declare_dram_parameter("BASIS", [24, 8 * NPIX], dt.float32, isOutput=False)
    OUT_d = nc.declare_dram_parameter("OUT", [M, NLOC], dt.float32, isOutput=True)

    with tile.TileContext(nc) as tc, ExitStack() as ctx:
        consts = ctx.enter_context(tc.tile_pool(name="consts", bufs=1))
        coeffp = ctx.enter_context(tc.tile_pool(name="coeffp", bufs=2))
        scratch = ctx.enter_context(tc.tile_pool(name="scratch", bufs=2))
        work = ctx.enter_context(tc.tile_pool(name="work", bufs=2))
        accp = ctx.enter_context(tc.tile_pool(name="accp", bufs=2))
        psum = ctx.enter_context(tc.tile_pool(name="psum", bufs=2, space="PSUM"))

        ident = consts.tile([128, 128], dt.float32)
        make_identity(nc, ident[:])
        PB = consts.tile([128, 10, NLOC], dt.float32)
        nc.sync.dma_start(out=PB[:].rearrange("p a b -> p (a b)"), in_=PB_d[:])
        TQ = consts.tile([128, NCHUNK, 10], dt.float32)
        nc.sync.dma_start(out=TQ[:].rearrange("p a b -> p (a b)"), in_=TQ_d[:])
        BAS = consts.tile([24, 8 * NPIX], dt.float32)
        nc.sync.dma_start(out=BAS[:], in_=BAS_d[:])

        def pb(q):
            i = ("x0", "x1", "y0", "y1", "cx", "cy", "ct", "st", "khw", "khh").index(q)
            return PB[:, i, :]

        def tq(j, q):
            i = ("x0", "x1", "y0", "y1", "cx", "cy", "ct", "st", "khw", "khh").index(q)
            return TQ[:, j, i:i + 1]

        for j in range(NCHUNK):
            # ---- coefficient slab C [128 m, 24 rows, 64 n] on GPSIMD ----
            C = coeffp.tile([128, 24, NLOC], dt.float32, tag="C")
            S = scratch.tile([128, 16, NLOC], dt.float32, tag="S")
            g = nc.vector

            def s(i):
                return S[:, i, :]

            if j == 0:
                # DVE instructions carry a single HW sync-wait slot, so the
                # first op after the two input DMAs may not wait on both DMA
                # sems at once.  Chain two single-wait ops; the WAW overlap
                # with s(0) orders the real first op after them with no waits.
                g.tensor_copy(s(0)[:, 1:2], PB[:, 0, 0:1])
                g.tensor_copy(s(0)[:, 0:1], TQ[:, 0, 0:1])

            g.tensor_scalar(s(0), pb("x0"), tq(j, "x0"), None, op.min)   # xmin
            g.tensor_scalar(s(1), pb("x1"), tq(j, "x1"), None, op.max)   # xmax
            g.tensor_scalar(s(2), pb("y0"), tq(j, "y0"), None, op.min)   # ymin
            g.tensor_scalar(s(3), pb("y1"), tq(j, "y1"), None, op.max)   # ymax
            g.tensor_tensor(s(4), s(1), s(0), op.subtract)               # sx
            g.tensor_tensor(s(5), s(3), s(2), op.subtract)               # sy
            g.tensor_tensor(s(6), s(0), pb("cx"), op.subtract)           # dxp
            g.tensor_tensor(s(7), s(2), pb("cy"), op.subtract)           # dyp
            # a0p = dxp*ctp + dyp*stp ; b0p = dyp*ctp - dxp*stp
            g.tensor_tensor(s(8), s(6), pb("ct"), op.mult)
            g.tensor_tensor(s(9), s(7), pb("st"), op.mult)
            g.tensor_tensor(s(9), s(8), s(9), op.add)                    # a0p
            g.tensor_tensor(s(8), s(7), pb("ct"), op.mult)
            g.tensor_tensor(s(10), s(6), pb("st"), op.mult)
            g.tensor_tensor(s(10), s(8), s(10), op.subtract)             # b0p

            def c(r):
                return C[:, r, :]

            # field A1 (const rows): P = khw_p - K*a0p ; Q = khw_p + K*a0p
            g.scalar_tensor_tensor(c(0), s(9), -K, pb("khw"), op.mult, op.add)
            g.scalar_tensor_tensor(c(12), s(9), K, pb("khw"), op.mult, op.add)
            # a1p = sx*ctp -> rows 1/13 ; a2p = sy*stp -> rows 2/14
            g.tensor_tensor(s(8), s(4), pb("ct"), op.mult)
            g.tensor_scalar(c(1), s(8), -K, None, op.mult)
            g.tensor_scalar(c(13), s(8), K, None, op.mult)
            g.tensor_tensor(s(8), s(5), pb("st"), op.mult)
            g.tensor_scalar(c(2), s(8), -K, None, op.mult)
            g.tensor_scalar(c(14), s(8), K, None, op.mult)
            # field B1 (rows 6-8/18-20; field order is A1,A2,B1,B2)
            g.scalar_tensor_tensor(c(6), s(10), -K, pb("khh"), op.mult, op.add)
            g.scalar_tensor_tensor(c(18), s(10), K, pb("khh"), op.mult, op.add)
            # b1p = -sx*stp: s8 = sx*stp -> P row = +K*s8, Q row = -K*s8
            g.tensor_tensor(s(8), s(4), pb("st"), op.mult)
            g.tensor_scalar(c(7), s(8), K, None, op.mult)
            g.tensor_scalar(c(19), s(8), -K, None, op.mult)
            # b2p = sy*ctp
            g.tensor_tensor(s(8), s(5), pb("ct"), op.mult)
            g.tensor_scalar(c(8), s(8), -K, None, op.mult)
            g.tensor_scalar(c(20), s(8), K, None, op.mult)
            # target box: dxt/dyt
            g.tensor_scalar(s(12), s(0), tq(j, "cx"), None, op.subtract)
            g.tensor_scalar(s(13), s(2), tq(j, "cy"), None, op.subtract)
            # a0t = dxt*ctt + dyt*stt
            g.tensor_scalar(s(8), s(12), tq(j, "ct"), None, op.mult)
            g.tensor_scalar(s(14), s(13), tq(j, "st"), None, op.mult)
            g.tensor_tensor(s(14), s(8), s(14), op.add)
            # b0t = dyt*ctt - dxt*stt
            g.tensor_scalar(s(8), s(13), tq(j, "ct"), None, op.mult)
            g.tensor_scalar(s(15), s(12), tq(j, "st"), None, op.mult)
            g.tensor_tensor(s(15), s(8), s(15), op.subtract)
            # field A2 const rows (rows 3-5/15-17)
            g.tensor_scalar(c(3), s(14), -K, tq(j, "khw"), op.mult, op.add)
            g.tensor_scalar(c(15), s(14), K, tq(j, "khw"), op.mult, op.add)
            # a1t = sx*ctt ; a2t = sy*stt
            g.tensor_scalar(s(8), s(4), tq(j, "ct"), None, op.mult)
            g.tensor_scalar(c(4), s(8), -K, None, op.mult)
            g.tensor_scalar(c(16), s(8), K, None, op.mult)
            g.tensor_scalar(s(8), s(5), tq(j, "st"), None, op.mult)
            g.tensor_scalar(c(5), s(8), -K, None, op.mult)
            g.tensor_scalar(c(17), s(8), K, None, op.mult)
            # field B2 const rows
            g.tensor_scalar(c(9), s(15), -K, tq(j, "khh"), op.mult, op.add)
            g.tensor_scalar(c(21), s(15), K, tq(j, "khh"), op.mult, op.add)
            # b1t = -sx*stt ; b2t = sy*ctt
            g.tensor_scalar(s(8), s(4), tq(j, "st"), None, op.mult)
            g.tensor_scalar(c(10), s(8), K, None, op.mult)
            g.tensor_scalar(c(22), s(8), -K, None, op.mult)
            g.tensor_scalar(s(8), s(5), tq(j, "ct"), None, op.mult)
            g.tensor_scalar(c(11), s(8), -K, None, op.mult)
            g.tensor_scalar(c(23), s(8), K, None, op.mult)

            Ssum = accp.tile([128, NLOC], dt.float32, tag="Ssum")
            Isum = accp.tile([128, NLOC], dt.float32, tag="Isum")

            # ---- main loop over the 64 predicted boxes of this core ----
            for n in range(NLOC):
                coeffT = psum.tile([24, 128], dt.float32, tag="coeffT")
                nc.tensor.transpose(coeffT[:], C[:, :, n], ident[:])
                lhsT = work.tile([24, 128], dt.float32, tag="lhsT")
                nc.vector.tensor_copy(lhsT[:], coeffT[:])

                fieldP = psum.tile([128, 4 * NPIX], dt.float32, tag="fields", bufs=3)
                fieldQ = psum.tile([128, 4 * NPIX], dt.float32, tag="fields", bufs=3)
                if j == 0 and n == 0:
                    # Warm the PE clock on the BAS DMA sem (single-wait LDW)
                    # before the first real matmul, which must wait on the
                    # DVE-written lhsT.  WAW into fieldP orders it first.
                    nc.tensor.transpose(fieldP[:, 0:24], BAS[0:24, 0:128], ident[0:24, 0:24])
                for q in range(2):
                    nc.tensor.matmul(
                        fieldP[:, q * 512:(q + 1) * 512],
                        lhsT[:], BAS[:, q * 512:(q + 1) * 512],
                        start=True, stop=True)
                for q in range(2):
                    nc.tensor.matmul(
                        fieldQ[:, q * 512:(q + 1) * 512],
                        lhsT[:], BAS[:, 1024 + q * 512:1024 + (q + 1) * 512],
                        start=True, stop=True)
                sigP = work.tile([128, 4 * NPIX], dt.bfloat16, tag="sigP")
                nc.scalar.activation(sigP[:], fieldP[:], AF.Sigmoid)
                sigQ = work.tile([128, 4 * NPIX], dt.bfloat16, tag="sigQ")
                nc.scalar.activation(sigQ[:], fieldQ[:], AF.Sigmoid)

                vmin = work.tile([128, 4, NPIX], dt.bfloat16, tag="vmin")
                nc.vector.tensor_tensor(
                    vmin[:].rearrange("p f q -> p (f q)"),
                    sigP[:], sigQ[:], op.min)

                vflat = vmin[:].rearrange("p f q -> p (f q)")
                Fp = work.tile([128, 2 * NPIX], dt.bfloat16, tag="Fp")
                nc.vector.tensor_mul(Fp[:], vflat[:, 0:2 * NPIX], vflat[:, 2 * NPIX:4 * NPIX])
                nc.vector.tensor_reduce(
                    Ssum[:, n:n + 1], Fp[:], mybir.AxisListType.X, op.add)
                F12 = work.tile([128, NPIX], dt.bfloat16, tag="F12")
                nc.vector.tensor_mul(F12[:], Fp[:, 0:NPIX], Fp[:, NPIX:2 * NPIX])
                nc.vector.tensor_reduce(
                    Isum[:, n:n + 1], F12[:], mybir.AxisListType.X, op.add)

            # ---- epilogue: piou = inter / (stot - inter + eps) ----
            union = scratch.tile([128, NLOC], dt.float32, tag="union")
            nc.vector.scalar_tensor_tensor(
                union[:], Isum[:], -1.0, Ssum[:], op.mult, op.add)
            nc.vector.tensor_scalar(union[:], union[:], float(EPS), None, op.add)
            rec = scratch.tile([128, NLOC], dt.float32, tag="rec")
            nc.vector.reciprocal(rec[:], union[:])
            piou = accp.tile([128, NLOC], dt.float32, tag="piou")
            nc.vector.tensor_tensor(piou[:], Isum[:], rec[:], op.mult)
            nc.sync.dma_start(out=OUT_d[j * 128:(j + 1) * 128, :], in_=piou[:])

    nc.finalize()
    return nc


def _get_compiled():
    if "nc" not in _cache:
        _cache["nc"] = _build_nc()
    return _cache["nc"]


def kernel(loc_p, loc_t, grid):
    from concourse.bass_utils import run_bass_kernel_spmd

    assert int(grid) == G
    loc_p = np.asarray(loc_p, dtype=np.float32)
    loc_t = np.asarray(loc_t, dtype=np.float32)
    basis, TQ, PBs = _host_constants(loc_p, loc_t)

    nc = _get_compiled()
    in_maps = [{"PB": PBs[c], "TQ": TQ, "BASIS": basis} for c in range(NC)]
    res = run_bass_kernel_spmd(nc, in_maps, core_ids=list(range(NC)))
    out = np.empty((N, M), dtype=np.float32)
    for c in range(NC):
        out[c * NLOC:(c + 1) * NLOC, :] = res.results[c]["OUT"].T
    return out



# revision 7
# speedup vs baseline: 574.8160x; 574.8160x over previous
"""PIoU (pixel-wise IoU) pairwise matrix kernel for Trainium2, 8 NeuronCores.

Math: for each pair (predicted box n, target box m) the reference samples a
16x16 grid of the joint AABB and evaluates a soft membership
F = sigmoid(k(w/2-|A|)) * sigmoid(k(h/2-|B|)) per box, where (A, B) are the
pixel offsets rotated into the box frame.  A and B are *affine* in the grid
coordinates (ug, uh), so k*(A, B) for all 256 pixels x 4 fields comes from
ONE K=12 matmul per 128 pairs against a constant [1, ug, uh] basis.  The
k*s/2 offset is folded into the sigmoid's per-partition bias:
    F-factor = sigmoid(-|k*d| + k*s/2)
(P-box extents are broadcast across partitions, so per-n they are a [128,1]
bias AP; T-box extents vary per partition, also a [128,1] AP.)

v3 pipeline per 128-pair x 1-n iteration (engines balanced ~1us each):
  PE      2 fp16 matmuls x 512 cols -> kd [128,1024] fp32 PSUM
  DVE     |kd| via tensor_scalar(abs_max, 0): PSUM -> SBUF fp16, 1 pass
  ACT     4x sigmoid(scale=-1, bias=k*s_f/2), one per field, fp16
  GPSIMD  2x scalar_tensor_tensor with accum_out:
            Fp=[F1|F2]=sigA*sigB (+Ssum), F12=F1*F2 (+Isum)
The box coordinates are pre-scaled by k on the host so the coefficient
build needs no extra scaling ops.  lhsT tiles for 3 consecutive n are
stacked at 32-partition offsets (PE base partitions are limited to
0/32/64) and produced by one [128,96] PE transpose per group; n is padded
64 -> 66 = 22*3 per core.

Sharding: N (predicted) axis split 8 ways; each core computes a [512m, 64n]
slab (output transposed on host).
"""

import numpy as np

N = 512
M = 512
G = 16
NPIX = G * G
K_SLOPE = np.float32(10.0)
EPS = np.float32(1e-6)
NC = 8
NLOC = N // NC  # 64 predicted boxes per core
NCHUNK = 4  # m-chunks of 128
NGRP = 22   # groups of 3 n (lhsT strips at partition 0/32/64)
NPAD = 3 * NGRP  # 66

_cache = {}


def _derived(b, k):
    # b: [K,5] float32 -> per-box derived quantities, coords pre-scaled by k
    cx, cy, w, h, t = (b[:, i].astype(np.float32) for i in range(5))
    c, s = np.cos(t).astype(np.float32), np.sin(t).astype(np.float32)
    hw = np.float32(0.5) * (w * np.abs(c) + h * np.abs(s))
    hh = np.float32(0.5) * (w * np.abs(s) + h * np.abs(c))
    return dict(
        cx=k * cx, cy=k * cy, ct=c, st=s,
        khw=(k * np.float32(0.5)) * w, khh=(k * np.float32(0.5)) * h,
        x0=k * (cx - hw), x1=k * (cx + hw), y0=k * (cy - hh), y1=k * (cy + hh),
    )


QORD = ("x0", "x1", "y0", "y1", "cx", "cy", "ct", "st", "khw", "khh")


def _host_constants(loc_p, loc_t):
    """Build per-core input arrays (all O(N+M) host work)."""
    u = ((np.arange(G, dtype=np.float32) + np.float32(0.5)) / np.float32(G))
    Ug = np.tile(u, G)      # pixel p = h*G+g -> u[g]
    Uh = np.repeat(u, G)    # -> u[h]

    # basis [12, NPIX*4]: field f (A1,A2,B1,B2) uses rows 3f..3f+2 = 1,Ug,Uh
    # on cols f*NPIX..(f+1)*NPIX.
    basis = np.zeros((12, 4 * NPIX), dtype=np.float32)
    for f in range(4):
        c0 = f * NPIX
        basis[3 * f + 0, c0:c0 + NPIX] = 1.0
        basis[3 * f + 1, c0:c0 + NPIX] = Ug
        basis[3 * f + 2, c0:c0 + NPIX] = Uh
    # BAS4 [128, 1024] fp16: basis replicated into 32-partition strips 0/32/64
    BAS4 = np.zeros((128, 4 * NPIX), dtype=np.float16)
    for i in range(3):
        BAS4[32 * i:32 * i + 12, :] = basis.astype(np.float16)

    T = _derived(loc_t, K_SLOPE)
    # TQ [128, 4 chunks, 10]: per-target quantities, m = j*128 + partition
    TQ = np.empty((128, NCHUNK, len(QORD)), dtype=np.float32)
    for qi, q in enumerate(QORD):
        TQ[:, :, qi] = T[q].reshape(NCHUNK, 128).T

    P = _derived(loc_p, K_SLOPE)
    PBs = []
    for c in range(NC):
        sl = slice(c * NLOC, (c + 1) * NLOC)
        pb = np.stack([P[q][sl] for q in QORD], axis=0)  # [10, 64]
        pb = np.concatenate([pb, pb[:, -1:].repeat(NPAD - NLOC, axis=1)], axis=1)
        PBs.append(np.broadcast_to(pb.reshape(1, 10 * NPAD), (128, 10 * NPAD)).copy())
    return BAS4, TQ.reshape(128, NCHUNK * len(QORD)), PBs


def _build_nc():
    from contextlib import ExitStack

    import concourse.bacc as bacc
    import concourse.tile as tile
    from concourse import mybir
    from concourse.masks import make_identity

    dt = mybir.dt
    op = mybir.AluOpType
    AF = mybir.ActivationFunctionType

    # Bacc (not raw Bass): its finalize() runs generate_event_semaphores,
    # which legalizes Tile's multi-wait sync_info down to <=1 wait per
    # hardware instruction.
    nc = bacc.Bacc(None, target_bir_lowering=False)
    PB_d = nc.declare_dram_parameter("PB", [128, 10 * NPAD], dt.float32, isOutput=False)
    TQ_d = nc.declare_dram_parameter("TQ", [128, NCHUNK * 10], dt.float32, isOutput=False)
    BAS_d = nc.declare_dram_parameter("BAS4", [128, 4 * NPIX], dt.float16, isOutput=False)
    OUT_d = nc.declare_dram_parameter("OUT", [M, NLOC], dt.float32, isOutput=True)

    with tile.TileContext(nc) as tc, ExitStack() as ctx:
        consts = ctx.enter_context(tc.tile_pool(name="consts", bufs=1))
        coeffp = ctx.enter_context(tc.tile_pool(name="coeffp", bufs=2))
        scratch = ctx.enter_context(tc.tile_pool(name="scratch", bufs=2))
        work = ctx.enter_context(tc.tile_pool(name="work", bufs=2))
        accp = ctx.enter_context(tc.tile_pool(name="accp", bufs=2))
        psum = ctx.enter_context(tc.tile_pool(name="psum", bufs=2, space="PSUM"))

        ident = consts.tile([128, 128], dt.float32)
        make_identity(nc, ident[:])
        PB = consts.tile([128, 10, NPAD], dt.float32)
        nc.sync.dma_start(out=PB[:].rearrange("p a b -> p (a b)"), in_=PB_d[:])
        TQ = consts.tile([128, NCHUNK, 10], dt.float32)
        nc.sync.dma_start(out=TQ[:].rearrange("p a b -> p (a b)"), in_=TQ_d[:])
        BAS = consts.tile([128, 4 * NPIX], dt.float16)
        nc.sync.dma_start(out=BAS[:], in_=BAS_d[:])

        def pb(q):
            # [128, 22, 3] view over the padded n axis
            return PB[:, QORD.index(q), :].rearrange("p (g i) -> p g i", g=NGRP)

        def pbn(q, n):
            # [128, 1] bias column for predicted box n (broadcast values)
            return PB[:, QORD.index(q), n:n + 1]

        def tq(j, q):
            return TQ[:, j, QORD.index(q)]

        for j in range(NCHUNK):
            # ---- coefficient slab C4 [128 m, 22 grp, 3 i, 32 r] ----
            # rows r=0..11 of (g,i) = K-scaled affine coefficients of pair
            # (m, n=3g+i): field f rows 3f..3f+2 = (d0, d_ug, d_uh);
            # rows 12..31 junk padding (never fed to the matmul).
            C4 = coeffp.tile([128, NGRP, 3, 32], dt.float32, tag="C4")
            S = scratch.tile([128, 12, NGRP, 3], dt.float32, tag="S")
            g = nc.vector

            def s(i):
                return S[:, i, :, :]

            if j == 0:
                # DVE instructions carry a single HW sync-wait slot, so the
                # first op after the two input DMAs may not wait on both DMA
                # sems at once.  Chain two single-wait ops; the WAW overlap
                # with s(0) orders the real first op after them with no waits.
                g.tensor_copy(S[:, 0, 0, 1:2], PB[:, 0, 0:1])
                g.tensor_copy(S[:, 0, 0, 0:1], TQ[:, 0, 0:1])

            def c(r):
                return C4[:, :, :, r]

            tj = lambda q: tq(j, q)[..., None]  # [128,1] scalar AP
            g.tensor_scalar(s(0), pb("x0"), tj("x0"), None, op.min)   # k*xmin
            g.tensor_scalar(s(1), pb("x1"), tj("x1"), None, op.max)   # k*xmax
            g.tensor_scalar(s(2), pb("y0"), tj("y0"), None, op.min)   # k*ymin
            g.tensor_scalar(s(3), pb("y1"), tj("y1"), None, op.max)   # k*ymax
            g.tensor_tensor(s(4), s(1), s(0), op.subtract)            # k*sx
            g.tensor_tensor(s(5), s(3), s(2), op.subtract)            # k*sy
            g.tensor_tensor(s(6), s(0), pb("cx"), op.subtract)        # k*dxp
            g.tensor_tensor(s(7), s(2), pb("cy"), op.subtract)        # k*dyp
            # A1: k*a0p = k*(dxp*ctp + dyp*stp); rows 0..2
            g.tensor_tensor(s(8), s(6), pb("ct"), op.mult)
            g.tensor_tensor(s(9), s(7), pb("st"), op.mult)
            g.tensor_tensor(c(0), s(8), s(9), op.add)
            g.tensor_tensor(c(1), s(4), pb("ct"), op.mult)            # k*sx*ctp
            g.tensor_tensor(c(2), s(5), pb("st"), op.mult)            # k*sy*stp
            # B1: k*b0p = k*(dyp*ctp - dxp*stp); rows 6..8
            g.tensor_tensor(s(8), s(7), pb("ct"), op.mult)
            g.tensor_tensor(s(9), s(6), pb("st"), op.mult)
            g.tensor_tensor(c(6), s(8), s(9), op.subtract)
            g.scalar_tensor_tensor(c(7), s(4), -1.0, pb("st"), op.mult, op.mult)
            g.tensor_tensor(c(8), s(5), pb("ct"), op.mult)            # k*sy*ctp
            # target box: k*dxt / k*dyt
            g.tensor_scalar(s(10), s(0), tj("cx"), None, op.subtract)
            g.tensor_scalar(s(11), s(2), tj("cy"), None, op.subtract)
            # A2: k*a0t; rows 3..5
            g.tensor_scalar(s(8), s(10), tj("ct"), None, op.mult)
            g.tensor_scalar(s(9), s(11), tj("st"), None, op.mult)
            g.tensor_tensor(c(3), s(8), s(9), op.add)
            g.tensor_scalar(c(4), s(4), tj("ct"), None, op.mult)      # k*sx*ctt
            g.tensor_scalar(c(5), s(5), tj("st"), None, op.mult)      # k*sy*stt
            # B2: k*b0t; rows 9..11
            g.tensor_scalar(s(8), s(11), tj("ct"), None, op.mult)
            g.tensor_scalar(s(9), s(10), tj("st"), None, op.mult)
            g.tensor_tensor(c(9), s(8), s(9), op.subtract)
            g.tensor_scalar(c(10), s(4), tj("st"), -1.0, op.mult, op.mult)
            g.tensor_scalar(c(11), s(5), tj("ct"), None, op.mult)     # k*sy*ctt

            Ssum = accp.tile([128, NPAD], dt.float32, tag="Ssum")
            Isum = accp.tile([128, NPAD], dt.float32, tag="Isum")

            # ---- main loop: 22 groups x 3 n ----
            for grp in range(NGRP):
                TP = psum.tile([128, 128], dt.float32, tag="tpose", bufs=2)
                if j == 0 and grp == 0:
                    # Warm the PE on the BAS DMA sem (single-wait) before the
                    # first real matmul, which must wait on the ACT-written
                    # lhsT.  WAW into TP orders the real transpose after it.
                    nc.tensor.matmul(
                        TP[:, 0:128], BAS[0:12, 0:128], BAS[0:12, 0:128],
                        start=True, stop=True)
                # transpose 3 n's coefficients at once: row 32i+r <- coeff r
                # of n=3*grp+i
                nc.tensor.transpose(
                    TP[0:96, 0:128],
                    C4[:, grp, :, :].rearrange("p a b -> p (a b)"),
                    ident[:])
                lhsT = work.tile([96, 128], dt.float16, tag="lhsT", bufs=2)
                nc.scalar.copy(lhsT[:], TP[0:96, 0:128])

                for i in range(3):
                    n = 3 * grp + i
                    F = psum.tile([128, 4 * NPIX], dt.float32, tag="fields", bufs=3)
                    lw = lhsT[32 * i:32 * i + 12, :]
                    for q in (0, 1):
                        nc.tensor.matmul(
                            F[:, q * 512:(q + 1) * 512],
                            lw, BAS[32 * i:32 * i + 12, q * 512:(q + 1) * 512],
                            start=True, stop=True)
                    # |k*d| : PSUM fp32 -> SBUF fp16 in one pass.  DVE does
                    # fields 0..2 via a size-1-axis reduce with
                    # apply_absolute_value (abs_max is not ISA-legal in
                    # TensorScalar); ACT takes field 3 (AF.Abs) to balance.
                    absk = work.tile([128, 4 * NPIX], dt.float16, tag="absk", bufs=2)
                    nc.vector.tensor_reduce(
                        absk[:, 0:3 * NPIX],
                        F[:, 0:3 * NPIX].rearrange("p (c one) -> p c one", one=1),
                        mybir.AxisListType.X, op.max, apply_absolute_value=True)
                    nc.scalar.activation(
                        absk[:, 3 * NPIX:4 * NPIX], F[:, 3 * NPIX:4 * NPIX], AF.Abs)
                    # sigmoid(k*s_f/2 - |k*d|) per field (bias = [128,1] AP)
                    sig = work.tile([128, 4 * NPIX], dt.float16, tag="sig", bufs=2)
                    for f, bias in enumerate((pbn("khw", n), tq(j, "khw")[..., None],
                                              pbn("khh", n), tq(j, "khh")[..., None])):
                        nc.scalar.activation(
                            sig[:, f * NPIX:(f + 1) * NPIX],
                            absk[:, f * NPIX:(f + 1) * NPIX],
                            AF.Sigmoid, bias=bias, scale=-1.0)
                    # Fp = [F1|F2] = [A1,A2]*[B1,B2]; Ssum[n] = sum(F1)+sum(F2)
                    # (DVE scalar_tensor_tensor fuses product + pixel sum;
                    # InstTensorTensorReduce crashes TRN2 hardware)
                    Fp = work.tile([128, 2 * NPIX], dt.float16, tag="Fp", bufs=2)
                    nc.vector.scalar_tensor_tensor(
                        Fp[:], sig[:, 0:2 * NPIX], 1.0, sig[:, 2 * NPIX:4 * NPIX],
                        op.mult, op.mult, accum_out=Ssum[:, n:n + 1])
                    # F12 = F1*F2 on GPSIMD; Isum[n] = sum(F12) on DVE
                    F12 = work.tile([128, NPIX], dt.float16, tag="F12", bufs=2)
                    nc.gpsimd.tensor_tensor(
                        F12[:], Fp[:, 0:NPIX], Fp[:, NPIX:2 * NPIX], op.mult)
                    nc.vector.tensor_reduce(
                        Isum[:, n:n + 1], F12[:], mybir.AxisListType.X, op.add)

            # ---- epilogue: piou = inter / (stot - inter + eps) ----
            union = scratch.tile([128, NPAD], dt.float32, tag="union")
            nc.vector.scalar_tensor_tensor(
                union[:], Isum[:], -1.0, Ssum[:], op.mult, op.add)
            nc.vector.tensor_scalar(union[:], union[:], float(EPS), None, op.add)
            rec = scratch.tile([128, NPAD], dt.float32, tag="rec")
            nc.vector.reciprocal(rec[:], union[:])
            piou = accp.tile([128, NPAD], dt.float32, tag="piou")
            nc.vector.tensor_tensor(piou[:], Isum[:], rec[:], op.mult)
            nc.sync.dma_start(out=OUT_d[j * 128:(j + 1) * 128, :], in_=piou[:, 0:NLOC])

    nc.finalize()
    return nc


def _get_compiled():
    if "nc" not in _cache:
        _cache["nc"] = _build_nc()
    return _cache["nc"]


def kernel(loc_p, loc_t, grid):
    from concourse.bass_utils import run_bass_kernel_spmd

    assert int(grid) == G
    loc_p = np.asarray(loc_p, dtype=np.float32)
    loc_t = np.asarray(loc_t, dtype=np.float32)
    BAS4, TQ, PBs = _host_constants(loc_p, loc_t)

    nc = _get_compiled()
    in_maps = [{"PB": PBs[c], "TQ": TQ, "BAS4": BAS4} for c in range(NC)]
    res = run_bass_kernel_spmd(nc, in_maps, core_ids=list(range(NC)))
    out = np.empty((N, M), dtype=np.float32)
    for c in range(NC):
        out[c * NLOC:(c + 1) * NLOC, :] = res.results[c]["OUT"].T
    return out


# revision 8
# speedup vs baseline: 6679.6778x; 11.6205x over previous
"""PIoU (pixel-wise IoU) pairwise matrix kernel for Trainium2, 8 NeuronCores.

Math: for each pair (predicted box n, target box m) the reference samples a
16x16 grid of the joint AABB and evaluates a soft membership
F = sigmoid(k(w/2-|A|)) * sigmoid(k(h/2-|B|)) per box, where (A, B) are the
pixel offsets rotated into the box frame.  A and B are *affine* in the grid
coordinates (ug, uh), so k*(A, B) for all 256 pixels x 4 fields comes from
ONE K=12 matmul per 128-pair tile against a constant [1, ug, uh] basis;
the k*s/2 offset folds into the sigmoid's per-partition bias:
    F-factor = sigmoid(-|k*d| + k*s/2)

NMS gating (the big one): with k=10 the sigmoid tails die within ~1px, so
any pair whose dilated AABBs (delta=2px) do not overlap has
piou < 1e-14 -- indistinguishable from 0 at fp32.  Only ~8% of the 512x512
pairs survive.  The host computes the O(N*M) AABB overlap mask (cheap
numpy), gathers the active pairs into 128-wide tiles (partition = pair),
and scatters the device results back into the zero matrix.  Each core gets
the active pairs of its 64 predicted boxes (~21 tiles); all cores are
padded to the same tile count T with duplicate pairs so one SPMD program
serves all 8.

Per 128-pair tile (pair quantities all live per-partition, so every
sigmoid bias is a [128,1] AP):
  PE      [128,32]->[32,128] coefficient transpose + 2 fp16 matmuls
          (512 cols each) -> k*d in [128,1024] fp32 PSUM
  DVE     |k*d| via size-1-axis reduce with apply_absolute_value
          (PSUM->SBUF fp16), then fused product+pixel-sum
          scalar_tensor_tensor -> Ssum, and the Isum reduce
  ACT     lhsT copy + 4x sigmoid(scale=-1, bias=k*s_f/2)
  GPSIMD  F12 = F1*F2 product
Ssum/Isum land in column t of [128,T] accumulators; the epilogue computes
piou = I/(S-I+eps) for all tiles and DMAs one [128,T] block out.
"""

import numpy as np

N = 512
M = 512
G = 16
NPIX = G * G
K_SLOPE = np.float32(10.0)
EPS = np.float32(1e-6)
NC = 8
NLOC = N // NC  # 64 predicted boxes per core
DELTA = np.float32(2.0)  # AABB dilation for the gating mask (px)

_cache = {}

# gathered per-pair quantity order: P-side 0..9, T-side 10..19
QORD = ("x0", "x1", "y0", "y1", "cx", "cy", "ct", "st", "khw", "khh")


def _derived(b, k):
    # b: [K,5] float32 -> per-box derived quantities, coords pre-scaled by k
    cx, cy, w, h, t = (b[:, i].astype(np.float32) for i in range(5))
    c, s = np.cos(t).astype(np.float32), np.sin(t).astype(np.float32)
    hw = np.float32(0.5) * (w * np.abs(c) + h * np.abs(s))
    hh = np.float32(0.5) * (w * np.abs(s) + h * np.abs(c))
    return dict(
        cx=k * cx, cy=k * cy, ct=c, st=s,
        khw=(k * np.float32(0.5)) * w, khh=(k * np.float32(0.5)) * h,
        x0=k * (cx - hw), x1=k * (cx + hw), y0=k * (cy - hh), y1=k * (cy + hh),
    )


def _basis():
    u = ((np.arange(G, dtype=np.float32) + np.float32(0.5)) / np.float32(G))
    Ug = np.tile(u, G)      # pixel p = h*G+g -> u[g]
    Uh = np.repeat(u, G)    # -> u[h]
    # [12, 1024]: field f (A1,A2,B1,B2) rows 3f..3f+2 = 1,Ug,Uh on its cols
    basis = np.zeros((12, 4 * NPIX), dtype=np.float16)
    for f in range(4):
        c0 = f * NPIX
        basis[3 * f + 0, c0:c0 + NPIX] = 1.0
        basis[3 * f + 1, c0:c0 + NPIX] = Ug.astype(np.float16)
        basis[3 * f + 2, c0:c0 + NPIX] = Uh.astype(np.float16)
    return basis


def _active_pairs(loc_p, loc_t):
    """Per-core gathered pair lists from the dilated-AABB overlap mask."""
    P = _derived(loc_p, np.float32(1.0))
    T = _derived(loc_t, np.float32(1.0))
    ov = ((np.minimum(P["x1"][:, None], T["x1"][None, :]) + DELTA
           >= np.maximum(P["x0"][:, None], T["x0"][None, :])) &
          (np.minimum(P["y1"][:, None], T["y1"][None, :]) + DELTA
           >= np.maximum(P["y0"][:, None], T["y0"][None, :])))
    pairs = []
    for c in range(NC):
        n_idx, m_idx = np.nonzero(ov[c * NLOC:(c + 1) * NLOC])
        pairs.append((n_idx.astype(np.int64) + c * NLOC, m_idx.astype(np.int64)))
    ntiles = max(1, max((len(n) + 127) // 128 for n, _ in pairs))
    return pairs, ntiles


def _host_constants(loc_p, loc_t, pairs, T):
    """CQ [128, 20, T] per core: gathered, K-prescaled pair quantities."""
    Pq = _derived(loc_p, K_SLOPE)
    Tq = _derived(loc_t, K_SLOPE)
    CQs = []
    for c in range(NC):
        n_idx, m_idx = pairs[c]
        cnt = len(n_idx)
        pad = T * 128 - cnt
        if cnt == 0:
            n_idx = np.array([c * NLOC], np.int64)
            m_idx = np.array([0], np.int64)
            cnt, pad = 1, T * 128 - 1
        n_full = np.concatenate([n_idx, np.repeat(n_idx[:1], pad)])
        m_full = np.concatenate([m_idx, np.repeat(m_idx[:1], pad)])
        CQ = np.empty((20, T * 128), dtype=np.float32)
        for qi, q in enumerate(QORD):
            CQ[qi] = Pq[q][n_full]
            CQ[10 + qi] = Tq[q][m_full]
        # [20, T*128] -> [128, 20, T]  (pair j = t*128 + p)
        CQs.append(np.ascontiguousarray(
            CQ.reshape(20, T, 128).transpose(2, 0, 1)).reshape(128, 20 * T))
    return CQs


def _build_nc(T):
    from contextlib import ExitStack

    import concourse.bacc as bacc
    import concourse.tile as tile
    from concourse import mybir
    from concourse.masks import make_identity

    dt = mybir.dt
    op = mybir.AluOpType
    AF = mybir.ActivationFunctionType

    # Bacc (not raw Bass): its finalize() runs generate_event_semaphores,
    # which legalizes Tile's multi-wait sync_info down to <=1 wait per
    # hardware instruction.
    nc = bacc.Bacc(None, target_bir_lowering=False)
    CQ_d = nc.declare_dram_parameter("CQ", [128, 20 * T], dt.float32, isOutput=False)
    BAS_d = nc.declare_dram_parameter("BAS", [12, 4 * NPIX], dt.float16, isOutput=False)
    OUT_d = nc.declare_dram_parameter("OUT", [128, T], dt.float32, isOutput=True)

    with tile.TileContext(nc) as tc, ExitStack() as ctx:
        consts = ctx.enter_context(tc.tile_pool(name="consts", bufs=1))
        work = ctx.enter_context(tc.tile_pool(name="work", bufs=2))
        psum = ctx.enter_context(tc.tile_pool(name="psum", bufs=2, space="PSUM"))

        ident = consts.tile([128, 128], dt.float32)
        make_identity(nc, ident[:])
        CQ = consts.tile([128, 20, T], dt.float32)
        nc.sync.dma_start(out=CQ[:].rearrange("p a b -> p (a b)"), in_=CQ_d[:])
        BAS = consts.tile([12, 4 * NPIX], dt.float16)
        nc.sync.dma_start(out=BAS[:], in_=BAS_d[:])

        def q(i):
            return CQ[:, i, :]

        # ---- coefficient slab C [128 pair, T, 32 r] ----
        # rows r=0..11 = K-scaled affine coefficients (field f rows 3f..3f+2);
        # rows 12..31 junk padding (never fed to the matmul).
        C = consts.tile([128, T, 32], dt.float32)
        S = consts.tile([128, 12, T], dt.float32)
        g = nc.vector

        def s(i):
            return S[:, i, :]

        def c(r):
            return C[:, :, r]

        g.tensor_tensor(s(0), q(0), q(10), op.min)    # k*xmin
        g.tensor_tensor(s(1), q(1), q(11), op.max)    # k*xmax
        g.tensor_tensor(s(2), q(2), q(12), op.min)    # k*ymin
        g.tensor_tensor(s(3), q(3), q(13), op.max)    # k*ymax
        g.tensor_tensor(s(4), s(1), s(0), op.subtract)   # k*sx
        g.tensor_tensor(s(5), s(3), s(2), op.subtract)   # k*sy
        g.tensor_tensor(s(6), s(0), q(4), op.subtract)   # k*dxp
        g.tensor_tensor(s(7), s(2), q(5), op.subtract)   # k*dyp
        # A1 rows 0..2: k*(dxp*ctp + dyp*stp), k*sx*ctp, k*sy*stp
        g.tensor_tensor(s(8), s(6), q(6), op.mult)
        g.tensor_tensor(s(9), s(7), q(7), op.mult)
        g.tensor_tensor(c(0), s(8), s(9), op.add)
        g.tensor_tensor(c(1), s(4), q(6), op.mult)
        g.tensor_tensor(c(2), s(5), q(7), op.mult)
        # B1 rows 6..8: k*(dyp*ctp - dxp*stp), -k*sx*stp, k*sy*ctp
        g.tensor_tensor(s(8), s(7), q(6), op.mult)
        g.tensor_tensor(s(9), s(6), q(7), op.mult)
        g.tensor_tensor(c(6), s(8), s(9), op.subtract)
        g.scalar_tensor_tensor(c(7), s(4), -1.0, q(7), op.mult, op.mult)
        g.tensor_tensor(c(8), s(5), q(6), op.mult)
        # target box offsets
        g.tensor_tensor(s(10), s(0), q(14), op.subtract)  # k*dxt
        g.tensor_tensor(s(11), s(2), q(15), op.subtract)  # k*dyt
        # A2 rows 3..5
        g.tensor_tensor(s(8), s(10), q(16), op.mult)
        g.tensor_tensor(s(9), s(11), q(17), op.mult)
        g.tensor_tensor(c(3), s(8), s(9), op.add)
        g.tensor_tensor(c(4), s(4), q(16), op.mult)
        g.tensor_tensor(c(5), s(5), q(17), op.mult)
        # B2 rows 9..11
        g.tensor_tensor(s(8), s(11), q(16), op.mult)
        g.tensor_tensor(s(9), s(10), q(17), op.mult)
        g.tensor_tensor(c(9), s(8), s(9), op.subtract)
        g.scalar_tensor_tensor(c(10), s(4), -1.0, q(17), op.mult, op.mult)
        g.tensor_tensor(c(11), s(5), q(16), op.mult)

        Ssum = consts.tile([128, T], dt.float32)
        Isum = consts.tile([128, T], dt.float32)

        # ---- main loop over pair tiles ----
        for t in range(T):
            TP = psum.tile([128, 128], dt.float32, tag="tpose", bufs=2)
            if t == 0:
                # Warm the PE on the BAS DMA sem (single-wait) before the
                # first real matmul, which must wait on the ACT-written lhsT.
                # WAW into TP orders the real transpose after it.
                nc.tensor.matmul(
                    TP[:, 0:128], BAS[0:12, 0:128], BAS[0:12, 0:128],
                    start=True, stop=True)
            nc.tensor.transpose(TP[0:32, 0:128], C[:, t, :], ident[:])
            lhsT = work.tile([32, 128], dt.float16, tag="lhsT", bufs=2)
            nc.scalar.copy(lhsT[:], TP[0:32, 0:128])

            F = psum.tile([128, 4 * NPIX], dt.float32, tag="fields", bufs=3)
            for qq in (0, 1):
                nc.tensor.matmul(
                    F[:, qq * 512:(qq + 1) * 512],
                    lhsT[0:12, :], BAS[0:12, qq * 512:(qq + 1) * 512],
                    start=True, stop=True)
            # |k*d| : PSUM fp32 -> SBUF fp16 in one DVE pass (abs_max is not
            # ISA-legal in TensorScalar; a size-1-axis reduce with
            # apply_absolute_value is)
            absk = work.tile([128, 4 * NPIX], dt.float16, tag="absk", bufs=2)
            nc.vector.tensor_reduce(
                absk[:], F[:].rearrange("p (c one) -> p c one", one=1),
                mybir.AxisListType.X, op.max, apply_absolute_value=True)
            # sigmoid(k*s_f/2 - |k*d|) per field; bias = gathered [128,1] AP
            sig = work.tile([128, 4 * NPIX], dt.float16, tag="sig", bufs=2)
            for f, bq in enumerate((8, 18, 9, 19)):  # khw_p, khw_t, khh_p, khh_t
                nc.scalar.activation(
                    sig[:, f * NPIX:(f + 1) * NPIX],
                    absk[:, f * NPIX:(f + 1) * NPIX],
                    AF.Sigmoid, bias=CQ[:, bq, t:t + 1], scale=-1.0)
            # Fp = [F1|F2] = [A1,A2]*[B1,B2]; Ssum[t] = sum(F1)+sum(F2)
            # (DVE scalar_tensor_tensor fuses product + pixel sum;
            # InstTensorTensorReduce crashes TRN2 hardware)
            Fp = work.tile([128, 2 * NPIX], dt.float16, tag="Fp", bufs=2)
            nc.vector.scalar_tensor_tensor(
                Fp[:], sig[:, 0:2 * NPIX], 1.0, sig[:, 2 * NPIX:4 * NPIX],
                op.mult, op.mult, accum_out=Ssum[:, t:t + 1])
            # F12 = F1*F2 on GPSIMD; Isum[t] = sum(F12) on DVE
            F12 = work.tile([128, NPIX], dt.float16, tag="F12", bufs=2)
            nc.gpsimd.tensor_tensor(
                F12[:], Fp[:, 0:NPIX], Fp[:, NPIX:2 * NPIX], op.mult)
            nc.vector.tensor_reduce(
                Isum[:, t:t + 1], F12[:], mybir.AxisListType.X, op.add)

        # ---- epilogue: piou = inter / (stot - inter + eps) ----
        union = consts.tile([128, T], dt.float32)
        nc.vector.scalar_tensor_tensor(
            union[:], Isum[:], -1.0, Ssum[:], op.mult, op.add)
        nc.vector.tensor_scalar(union[:], union[:], float(EPS), None, op.add)
        rec = consts.tile([128, T], dt.float32)
        nc.vector.reciprocal(rec[:], union[:])
        piou = consts.tile([128, T], dt.float32)
        nc.vector.tensor_tensor(piou[:], Isum[:], rec[:], op.mult)
        nc.sync.dma_start(out=OUT_d[:], in_=piou[:])

    nc.finalize()
    return nc


def _get_compiled(T):
    if T not in _cache:
        _cache[T] = _build_nc(T)
    return _cache[T]


def kernel(loc_p, loc_t, grid):
    from concourse.bass_utils import run_bass_kernel_spmd

    assert int(grid) == G
    loc_p = np.asarray(loc_p, dtype=np.float32)
    loc_t = np.asarray(loc_t, dtype=np.float32)
    pairs, T = _active_pairs(loc_p, loc_t)
    CQs = _host_constants(loc_p, loc_t, pairs, T)
    BAS = _basis()

    nc = _get_compiled(T)
    in_maps = [{"CQ": CQs[c], "BAS": BAS} for c in range(NC)]
    res = run_bass_kernel_spmd(nc, in_maps, core_ids=list(range(NC)))
    out = np.zeros((N, M), dtype=np.float32)
    for c in range(NC):
        n_idx, m_idx = pairs[c]
        cnt = len(n_idx)
        vals = res.results[c]["OUT"].T.reshape(-1)[:cnt]  # pair j = t*128+p
        out[n_idx, m_idx] = vals
    return out
